# revision 60
# baseline (speedup 1.0000x reference)
"""T5-style relative-position-bias attention on 8 TRN2 NeuronCores.

Full-input contract: kernel(**inputs) takes the unsharded tensors and
returns the full [2, 2048, 1024] output.

Sharding: 16 heads / 8 cores = 2 heads per core, both batches on every
core (data stays identical; only weight shards differ). Each core
computes its partial output projection (its heads' contribution to the
full [B, S, D] output); the host sums the 8 partials.

Schedule (v2): eight single (b, q-block) attention streams run back to
back, software-pipelined one kp ahead so the PE never waits on the
scalar-engine exps. All projection chains (both batches) and the Wo
matmuls are woven uniformly into the streams' kp slots as PE filler.
PSUM: scores 2x[128,1024] (4 banks) + ctx 2x[65,512] (2) + proj/wo
2x[128,512] (2) = 8 banks.
"""

import math
import sys

sys.path.insert(0, "/opt/trn_rl_repo")

import numpy as np
import ml_dtypes

BF16 = ml_dtypes.bfloat16

B, S, D, H, HD = 2, 2048, 1024, 16, 64
N_CORES = 8
HEADS_PER_CORE = H // N_CORES  # 2
SCALING = HD ** (-0.5)
NUM_BUCKETS = 32
MAX_DISTANCE = 128

# q-block = 512 columns of the (transposed) score tile; k-tile = 128 rows.
QB = 512
KT = 128
N_QB = S // QB  # 4
N_KT = S // KT  # 16
# near-diagonal offsets m = kt - 4*qb for which bias varies inside the tile
NEAR_MS = list(range(-1, 5))  # -1..4


def _bucket_np(d):
    """Port of reference._relative_position_bucket (bidirectional), float32."""
    nb = NUM_BUCKETS // 2  # 16
    rb = (d > 0).astype(np.int32) * nb
    ad = np.abs(d)
    max_exact = nb // 2  # 8
    is_small = ad < max_exact
    rp = np.maximum(ad, 1).astype(np.float32)
    ril = max_exact + (
        np.log(rp / np.float32(max_exact))
        / np.float32(math.log(MAX_DISTANCE / max_exact))
        * np.float32(nb - max_exact)
    ).astype(np.int32)
    ril = np.minimum(ril, nb - 1)
    return rb + np.where(is_small, ad, ril)


def _near_bucket_tables():
    """Bucket index tile [128, 512] for each near offset m (head-independent)."""
    tables = {}
    p = np.arange(KT)[:, None]
    j = np.arange(QB)[None, :]
    for m in NEAR_MS:
        d = KT * m + p - j  # d = k - q
        tables[m] = _bucket_np(d)
    return tables


_NEAR_BUCKETS = _near_bucket_tables()


def _prep_core_inputs(c, hidden_states, Wq, Wk, Wv, Wo, rel_emb, xt_by_batch):
    """Batch-split sharding: core c handles batch c//4 and head group c%4
    (4 heads, as two head-pairs hp=0,1). Halves per-core xt traffic vs
    replicating both batches."""
    g = c % 4
    rows = slice(256 * g, 256 * (g + 1))

    def wsl(W):  # [(hp*8+dt), 128, 128] chunks of W[rows].T
        wt = np.ascontiguousarray(W[rows].T)  # [1024, 256]
        return np.ascontiguousarray(
            wt.reshape(8, 128, 2, 128).transpose(2, 0, 1, 3)
            .reshape(16, 128, 128)
        ).astype(BF16)

    wqt, wkt, wvt = wsl(Wq), wsl(Wk), wsl(Wv)
    # [2, 128, 1024]: head-pair hp's slice of Wo
    wot = np.ascontiguousarray(
        Wo[:, rows].T.reshape(2, 128, 1024)
    ).astype(BF16)

    # exp(bias) tiles for 4 heads: [(hp*2+hl)*6 + mi, 128, 512]
    etab = np.empty((4 * len(NEAR_MS), KT, QB), dtype=BF16)
    bfar = np.empty((8,), dtype=np.float32)
    for hp in range(2):
        for hl in range(2):
            h = 4 * g + 2 * hp + hl
            for mi, m in enumerate(NEAR_MS):
                etab[(hp * 2 + hl) * len(NEAR_MS) + mi] = np.exp(
                    rel_emb[_NEAR_BUCKETS[m], h].astype(np.float32)
                ).astype(BF16)
            bfar[(hp * 2 + hl) * 2 + 0] = rel_emb[15, h]  # far negative
            bfar[(hp * 2 + hl) * 2 + 1] = rel_emb[31, h]  # far positive
    bfar_t = np.tile(bfar[None, :], (128, 1)).astype(np.float32)
    bfarexp_t = np.exp(bfar_t).astype(np.float32)

    return {
        "xt": xt_by_batch[c // 4],
        "wqt": wqt,
        "wkt": wkt,
        "wvt": wvt,
        "wot": wot,
        "etab": etab,
        "bfar": bfar_t,
        "bfarexp": bfarexp_t,
    }


_PROGRAM_CACHE = {}

# etab DMA order: tiles needed by stream (0,0)'s first kps go first
_ETAB_ORDER = ([1, 2, 7, 8, 3, 4, 9, 10, 5, 11, 0, 6]
               + [13, 14, 19, 20, 15, 16, 21, 22, 17, 23, 12, 18])


def _build_program():
    if "nc" in _PROGRAM_CACHE:
        return _PROGRAM_CACHE["nc"]

    from contextlib import ExitStack

    import concourse.bass as bass
    import concourse.tile as tile
    from concourse import bacc, mybir

    f32 = mybir.dt.float32
    bf16 = mybir.dt.bfloat16
    Exp = mybir.ActivationFunctionType.Exp

    nc = bacc.Bacc("TRN2", target_bir_lowering=False, debug=False,
                   num_devices=N_CORES)

    xt_d = nc.dram_tensor("xt", [N_QB * 8, 128, QB], bf16,
                          kind="ExternalInput").ap()
    wqt_d = nc.dram_tensor("wqt", [16, 128, 128], bf16, kind="ExternalInput").ap()
    wkt_d = nc.dram_tensor("wkt", [16, 128, 128], bf16, kind="ExternalInput").ap()
    wvt_d = nc.dram_tensor("wvt", [16, 128, 128], bf16, kind="ExternalInput").ap()
    wot_d = nc.dram_tensor("wot", [2, 128, 1024], bf16, kind="ExternalInput").ap()
    etab_d = nc.dram_tensor("etab", [24, 128, 512], bf16, kind="ExternalInput").ap()
    bfar_d = nc.dram_tensor("bfar", [128, 8], f32, kind="ExternalInput").ap()
    bfarexp_d = nc.dram_tensor("bfarexp", [128, 8], f32,
                               kind="ExternalInput").ap()
    # per-head-pair partials of this core's batch; host sums them
    out_d = nc.dram_tensor("out", [2, S, D], bf16, kind="ExternalOutput").ap()

    VSLOT = 2 * 65  # [VA | 1 | VB | 1] per (b, kt)

    with tile.TileContext(nc) as tc, ExitStack() as ctx:
        const = ctx.enter_context(tc.tile_pool(name="const", bufs=1))

        xt_sb = const.tile([128, 8 * S], bf16, tag="xt")
        wq_sb = const.tile([128, 16 * 128], bf16, tag="wq")
        wk_sb = const.tile([128, 16 * 128], bf16, tag="wk")
        wv_sb = const.tile([128, 16 * 128], bf16, tag="wv")
        etab_sb = const.tile([128, 24 * 512], bf16, tag="etab")
        bfar_sb = const.tile([128, 8], f32, tag="bfar")
        bfarexp_sb = const.tile([128, 8], f32, tag="bfarexp")
        wot_sb = const.tile([128, 2 * 1024], bf16, tag="wot")

        # DMA priority order: weights, then b0 xt qb0-1, the small bias
        # tables, b0 xt qb2-3, etab, wot, then all of b1's xt.
        def emit_w_dma(hps):
            for hp in hps:
                for w_sb, w_d in ((wk_sb, wkt_d), (wq_sb, wqt_d),
                                  (wv_sb, wvt_d)):
                    for i in range(8):
                        ci = hp * 8 + i
                        eng = (nc.sync, nc.gpsimd, nc.scalar)[i % 3]
                        eng.dma_start(
                            w_sb[:, 128 * ci : 128 * (ci + 1)], w_d[ci])

        def emit_xt_dma(qbs, n_engs=2):
            # scalar may only carry early transfers: later issues on its
            # queue would block the exp stream behind DMA-ring backpressure
            engs = (nc.sync, nc.gpsimd, nc.scalar)[:n_engs]
            for qb in qbs:
                for dt in range(8):
                    eng = engs[dt % n_engs]
                    ci = qb * 8 + dt
                    col = dt * S + qb * QB
                    eng.dma_start(xt_sb[:, col : col + QB], xt_d[ci])

        emit_w_dma((0,))
        emit_xt_dma((0, 1), n_engs=3)
        nc.scalar.dma_start(bfar_sb[:], bfar_d[:])
        nc.scalar.dma_start(bfarexp_sb[:], bfarexp_d[:])
        for j, i in enumerate(_ETAB_ORDER[:4]):
            eng = (nc.sync, nc.gpsimd, nc.scalar)[j % 3]
            eng.dma_start(etab_sb[:, 512 * i : 512 * (i + 1)], etab_d[i])
        emit_xt_dma((2, 3))
        emit_w_dma((1,))
        for j, i in enumerate(_ETAB_ORDER[4:]):
            eng = (nc.sync, nc.gpsimd)[j % 2]
            eng.dma_start(etab_sb[:, 512 * i : 512 * (i + 1)], etab_d[i])
        for hp in range(2):
            nc.sync.dma_start(wot_sb[:, hp * 1024 : (hp + 1) * 1024],
                              wot_d[hp])

        qt_sb = const.tile([128, B * S], bf16, tag="qt")
        kt_sb = const.tile([128, B * S], bf16, tag="kt")
        v_sb = const.tile([128, B * N_KT * VSLOT], bf16, tag="v")
        # ones columns at offsets 64/129 of each VSLOT: set the whole tile
        # to 1.0 once; the V projection copies overwrite the data columns.
        nc.vector.memset(v_sb[:], 1.0)

        def cls(m):
            if m <= -2:
                return 0  # far negative
            if m >= 5:
                return 1  # far positive
            return 2  # near

        psum = ctx.enter_context(
            tc.tile_pool(name="psum", bufs=2, space="PSUM"))
        utp = ctx.enter_context(tc.tile_pool(name="utp", bufs=8))
        ostg = ctx.enter_context(tc.tile_pool(name="ostg", bufs=4))
        nrm = ctx.enter_context(tc.tile_pool(name="nrm", bufs=2))

        def emit_qk_chain(b, qb, wi):
            w_sb, dst = ((wq_sb, qt_sb), (wk_sb, kt_sb))[wi]
            ps = psum.tile([128, 512], f32, tag="proj",
                           name=f"pj_{b}_{qb}_{wi}")
            for dt in range(8):
                ci = b * 8 + dt
                nc.tensor.matmul(
                    ps[:],
                    lhsT=w_sb[:, 128 * ci : 128 * (ci + 1)],
                    rhs=xt_sb[:, dt * S + qb * QB :
                              dt * S + qb * QB + QB],
                    start=(dt == 0), stop=(dt == 7),
                )
            nc.vector.tensor_copy(
                dst[:, b * S + qb * QB : b * S + qb * QB + QB], ps[:]
            )

        def emit_v_chain(b, st):
            ps = psum.tile([128, 128], f32, tag="proj",
                           name=f"vp_{b}_{st}")
            for dt in range(8):
                ci = b * 8 + dt
                nc.tensor.matmul(
                    ps[:],
                    lhsT=xt_sb[:, dt * S + st * KT :
                               dt * S + st * KT + KT],
                    rhs=wv_sb[:, 128 * ci : 128 * (ci + 1)],
                    start=(dt == 0), stop=(dt == 7),
                )
            base = (b * N_KT + st) * VSLOT
            nc.vector.tensor_copy(v_sb[:, base : base + 64], ps[:, 0:64])
            nc.vector.tensor_copy(v_sb[:, base + 65 : base + 129],
                                  ps[:, 64:128])

        def emit_scores(b, qb, kp):
            """Score matmuls + exp (+ bias mul) for one kp of one stream.
            Returns per-hl source list for the AV matmuls."""
            m0 = 2 * kp - 4 * qb
            m1 = m0 + 1
            c0, c1 = cls(m0), cls(m1)
            out = []
            for hl in range(2):
                sct = psum.tile([128, 1024], f32, tag="sc",
                                name=f"sc_{b}_{qb}_{kp}_{hl}")
                for half in range(2):
                    kt = 2 * kp + half
                    nc.tensor.matmul(
                        sct[:, 512 * half : 512 * (half + 1)],
                        lhsT=kt_sb[64 * hl : 64 * (hl + 1),
                                   b * S + kt * KT : b * S + kt * KT + KT],
                        rhs=qt_sb[64 * hl : 64 * (hl + 1),
                                  b * S + qb * QB : b * S + qb * QB + QB],
                        start=True, stop=True,
                    )
                ut = utp.tile([128, 1024], bf16, tag="ut",
                              name=f"ut_{b}_{qb}_{kp}_{hl}")
                srcs = [(ut, 0), (ut, 512)]
                hg = b * 2 + hl  # head index within this core's 4 heads
                if c0 == c1 and c0 != 2:
                    nc.scalar.activation(
                        ut[:], sct[:], Exp,
                        bias=bfar_sb[:, 2 * hg + c0 : 2 * hg + c0 + 1],
                        scale=SCALING,
                    )
                else:
                    nc.scalar.activation(
                        ut[:], sct[:], Exp, bias=0.0, scale=SCALING
                    )
                    ut2 = utp.tile([128, 1024], bf16, tag="ut2",
                                   name=f"ut2_{b}_{qb}_{kp}_{hl}")
                    if c0 == c1 == 2:
                        ei = (hg * 6 + (m0 + 1)) * 512
                        nc.vector.tensor_mul(
                            ut2[:], ut[:], etab_sb[:, ei : ei + 1024]
                        )
                        srcs = [(ut2, 0), (ut2, 512)]
                    else:
                        for half, (m, cc) in enumerate(((m0, c0), (m1, c1))):
                            usl = ut[:, 512 * half : 512 * (half + 1)]
                            osl = ut2[:, 512 * half : 512 * (half + 1)]
                            if cc == 2:
                                ei = (hg * 6 + (m + 1)) * 512
                                nc.vector.tensor_mul(
                                    osl, usl, etab_sb[:, ei : ei + 512]
                                )
                            else:
                                col = 2 * hg + cc
                                nc.vector.tensor_scalar_mul(
                                    osl, usl, bfarexp_sb[:, col : col + 1]
                                )
                            srcs[half] = (ut2, 512 * half)
                out.append(srcs)
            return out

        def emit_av(b, qb, kp, srcs2, ctxs, hl):
            for half in range(2):
                kt = 2 * kp + half
                base = (b * N_KT + kt) * VSLOT + 65 * hl
                stile, soff = srcs2[hl][half]
                nc.tensor.matmul(
                    ctxs[hl][:],
                    lhsT=v_sb[:, base : base + 65],
                    rhs=stile[:, soff : soff + 512],
                    start=(kt == 0), stop=(kt == N_KT - 1),
                )

        lct_of = {}

        def emit_norm(si):
            b, qb = STREAMS[si]
            ctxs = ctx_of[si]
            lct = nrm.tile([128, 512], bf16, tag="lct", bufs=2,
                           name=f"lct_{b}_{qb}")
            rzfs, rzbs = [], []
            for hl in range(2):
                rz = nrm.tile([1, 512], f32, tag=f"rz{hl}", bufs=2,
                              name=f"rz_{b}_{qb}_{hl}")
                nc.vector.tensor_copy(rz[0:1, :], ctxs[hl][64:65, :])
                rzf = nrm.tile([1, 512], f32, tag=f"rzf{hl}", bufs=2,
                               name=f"rzf_{b}_{qb}_{hl}")
                nc.vector.reciprocal_approx_fast(
                    out=rzf[0:1, :], in_=rz[0:1, :]
                )
                rzfs.append(rzf)
            for hl in range(2):
                rzb = nrm.tile([64, 512], f32, tag=f"rzb{hl}", bufs=2,
                               name=f"rzb_{b}_{qb}_{hl}")
                nc.gpsimd.partition_broadcast(
                    rzb[:], rzfs[hl][0:1, :], channels=64
                )
                rzbs.append(rzb)
            for hl in range(2):
                nc.vector.tensor_mul(
                    lct[64 * hl : 64 * (hl + 1), :],
                    ctxs[hl][0:64, :], rzbs[hl][:])
            lct_of[si] = lct

        def emit_wo(si, st, tail=False):
            b, qb = STREAMS[si]
            lct = lct_of[si]
            ot = ostg.tile([128, 1024], bf16, tag="ot",
                           name=f"ot_{b}_{qb}_{st}")
            for nh in range(2):
                wo_ps = psum.tile([128, 512], f32, tag="proj",
                                  name=f"wo_{b}_{qb}_{st}_{nh}")
                nc.tensor.matmul(
                    wo_ps[:],
                    lhsT=lct[:, st * 128 : (st + 1) * 128],
                    rhs=wot_sb[:, b * 1024 + nh * 512 :
                               b * 1024 + (nh + 1) * 512],
                    start=True, stop=True,
                )
                # in the tail the exps are done, so the scalar engine is
                # free to halve the PSUM-evacuation latency
                if tail and nh == 0:
                    nc.scalar.copy(ot[:, 0:512], wo_ps[:])
                else:
                    nc.vector.tensor_copy(ot[:, nh * 512 : (nh + 1) * 512],
                                          wo_ps[:])
            srow = qb * QB + st * 128
            eng = (nc.gpsimd, nc.sync, nc.scalar)[st % 3] if tail else \
                (nc.gpsimd, nc.sync)[st % 2]
            eng.dma_start(out_d[b, srow : srow + 128, :], ot[:])

        # ---- schedule ----
        STREAMS = [(0, 0), (0, 1), (0, 2), (0, 3),
                   (1, 0), (1, 1), (1, 2), (1, 3)]
        ctx_of = {}

        def qk(b, qb, wi):
            return lambda: emit_qk_chain(b, qb, wi)

        def vch(b, st):
            return lambda: emit_v_chain(b, st)

        def wo(si, st):
            return lambda: emit_wo(si, st)

        def dum(n):
            # p-state filler: keeps the PE streaming through DMA-paced
            # stretches so the clock stays ramped; results are discarded
            def f():
                d = psum.tile([128, 512], f32, tag="proj", name="dum")
                for _ in range(n):
                    nc.tensor.matmul(
                        d[:], lhsT=qt_sb[:, 0:128], rhs=qt_sb[:, 0:512],
                        start=True, stop=True,
                    )
            return f

        fill = {s: {kp: [] for kp in range(8)} for s in range(8)}
        # stream 0: rest of batch-0 projections, need-ordered
        fill[0][0] = [qk(0, 1, 1), vch(0, 2), vch(0, 3)]
        fill[0][1] = [vch(0, 4), vch(0, 5)]
        fill[0][2] = [qk(0, 2, 1), vch(0, 6), vch(0, 7)]
        fill[0][3] = [vch(0, 8), vch(0, 9)]
        fill[0][4] = [qk(0, 3, 1), vch(0, 10), vch(0, 11)]
        fill[0][5] = [vch(0, 12), vch(0, 13), qk(0, 1, 0)]
        fill[0][6] = [vch(0, 14), vch(0, 15)]
        fill[0][7] = [qk(0, 2, 0)]
        # streams 1-3: batch-1 projections + prev stream's norm/wo
        # Wo filler split: st0/st1 land mid-next-stream; st2/st3 land at the
        # FOLLOWING stream's transition slots (lct long since ready there,
        # so they give the PE real work while the fresh norm chain runs).
        fill[1][0] = [qk(0, 3, 0)]
        fill[1][1] = [qk(1, 0, 1)]
        fill[1][2] = [wo(0, 0), vch(1, 0)]
        fill[1][3] = [wo(0, 1), vch(1, 1)]
        fill[1][4] = [wo(0, 2), qk(1, 0, 0)]
        fill[1][5] = [wo(0, 3), vch(1, 2)]
        fill[1][6] = [qk(1, 1, 1)]
        fill[1][7] = [vch(1, 3)]
        fill[2][0] = [vch(1, 4)]
        fill[2][1] = [qk(1, 2, 1), vch(1, 5)]
        fill[2][2] = [wo(1, 0), vch(1, 6)]
        fill[2][3] = [wo(1, 1), vch(1, 7)]
        fill[2][4] = [wo(1, 2), vch(1, 8)]
        fill[2][5] = [wo(1, 3), vch(1, 9)]
        fill[2][6] = [qk(1, 3, 1), vch(1, 10)]
        fill[2][7] = [vch(1, 11)]
        fill[3][0] = [vch(1, 12)]
        fill[3][1] = [vch(1, 13)]
        fill[3][2] = [wo(2, 0), vch(1, 14)]
        fill[3][3] = [wo(2, 1), vch(1, 15)]
        fill[3][4] = [wo(2, 2)]
        fill[3][5] = [wo(2, 3)]
        for s in range(4, 8):
            if s < 7:
                fill[s][0] = [qk(1, s - 3, 0)]
            for st in range(4):
                fill[s][2 + st].append(wo(s - 1, st))
        # streams 4-7 run out of real projection filler: their PE load/slot
        # (~1.9us) sits just under the scalar exp pace (~2.15us), so every
        # slot ends in a micro-gap that resets the PE clock ramp and the
        # attention matmuls drop to mid p-state. Two dummy matmuls per
        # empty slot (~0.43us) pack the slots to the scalar pace and keep
        # the clock pinned at max.
        fill[3][6].append(dum(2))
        fill[3][7].append(dum(2))
        for s in range(4, 8):
            fill[s][1].append(dum(2))
            fill[s][6].append(dum(2))
            fill[s][7].append(dum(2))
        fill[7][0].append(dum(3))

        # prologue: minimum chains for stream (0,0) kp0
        emit_qk_chain(0, 0, 1)  # k
        emit_qk_chain(0, 0, 0)  # q
        emit_v_chain(0, 0)
        emit_v_chain(0, 1)

        all_slots = [(si, b, qb, kp)
                     for si, (b, qb) in enumerate(STREAMS)
                     for kp in range(8)]
        pend = emit_scores(0, 0, 0)
        for i, (si, b, qb, kp) in enumerate(all_slots):
            cur = pend
            if i + 1 < len(all_slots):
                si2, b2, qb2, kp2 = all_slots[i + 1]
                pend = emit_scores(b2, qb2, kp2)
            if kp == 0:
                ctx_of[si] = {
                    hl: psum.tile([65, 512], f32, tag="ctx",
                                  name=f"ctx_{b}_{qb}_{hl}")
                    for hl in range(2)
                }
            for f in fill[si][kp]:
                f()
            emit_av(b, qb, kp, cur, ctx_of[si], 0)
            emit_av(b, qb, kp, cur, ctx_of[si], 1)
            # start the softmax-normalization chain the moment the
            # stream's ctx accumulation completes
            if kp == 7:
                emit_norm(si)

        # tail: final output projections; a few dummy matmuls bridge the
        # norm(7) latency so the final Wo matmuls run at full clock
        dum(6)()
        for st in range(4):
            emit_wo(7, st, tail=True)

    nc.compile()
    _PROGRAM_CACHE["nc"] = nc
    return nc


def run(inputs, trace=False, trace_kwargs=None):
    """Returns (full_output, BassKernelResults)."""
    from concourse.bass_utils import run_bass_kernel_spmd

    hidden_states = np.asarray(inputs["hidden_states"], dtype=np.float32)
    Wq = np.asarray(inputs["Wq"], dtype=np.float32)
    Wk = np.asarray(inputs["Wk"], dtype=np.float32)
    Wv = np.asarray(inputs["Wv"], dtype=np.float32)
    Wo = np.asarray(inputs["Wo"], dtype=np.float32)
    rel_emb = np.asarray(inputs["rel_emb"], dtype=np.float32)

    xt = np.ascontiguousarray(hidden_states.transpose(0, 2, 1))  # [B, D, S]
    # per-batch [(qb, dt), 128, QB] chunk layout matching the kernel's DMA
    xt_by_batch = [
        np.ascontiguousarray(
            xt[b].reshape(8, 128, S // QB, QB).transpose(2, 0, 1, 3)
            .reshape(N_QB * 8, 128, QB)
        ).astype(BF16)
        for b in range(B)
    ]

    nc = _build_program()
    in_maps = [
        _prep_core_inputs(c, hidden_states, Wq, Wk, Wv, Wo, rel_emb,
                          xt_by_batch)
        for c in range(N_CORES)
    ]
    res = run_bass_kernel_spmd(
        nc, in_maps, list(range(N_CORES)), trace=trace,
        **(trace_kwargs or {}),
    )
    out = np.zeros((B, S, D), dtype=np.float32)
    for c in range(N_CORES):
        p = res.results[c]["out"].astype(np.float32)
        out[c // 4] += p[0] + p[1]
    return out, res


def kernel(**inputs):
    out, _ = run(inputs)
    return out


# revision 61
# speedup vs baseline: 1.0409x; 1.0409x over previous
"""T5-style relative-position-bias attention on 8 TRN2 NeuronCores.

Full-input contract: kernel(**inputs) takes the unsharded tensors and
returns the full [2, 2048, 1024] output.

Sharding: 16 heads / 8 cores = 2 heads per core, both batches on every
core (data stays identical; only weight shards differ). Each core
computes its partial output projection (its heads' contribution to the
full [B, S, D] output); the host sums the 8 partials.

Schedule (v2): eight single (b, q-block) attention streams run back to
back, software-pipelined one kp ahead so the PE never waits on the
scalar-engine exps. All projection chains (both batches) and the Wo
matmuls are woven uniformly into the streams' kp slots as PE filler.
PSUM: scores 2x[128,1024] (4 banks) + ctx 2x[65,512] (2) + proj/wo
2x[128,512] (2) = 8 banks.
"""

import math
import sys

sys.path.insert(0, "/opt/trn_rl_repo")

import numpy as np
import ml_dtypes

BF16 = ml_dtypes.bfloat16

B, S, D, H, HD = 2, 2048, 1024, 16, 64
N_CORES = 8
HEADS_PER_CORE = H // N_CORES  # 2
SCALING = HD ** (-0.5)
NUM_BUCKETS = 32
MAX_DISTANCE = 128

# q-block = 512 columns of the (transposed) score tile; k-tile = 128 rows.
QB = 512
KT = 128
N_QB = S // QB  # 4
N_KT = S // KT  # 16
# near-diagonal offsets m = kt - 4*qb for which bias varies inside the tile
NEAR_MS = list(range(-1, 5))  # -1..4


def _bucket_np(d):
    """Port of reference._relative_position_bucket (bidirectional), float32."""
    nb = NUM_BUCKETS // 2  # 16
    rb = (d > 0).astype(np.int32) * nb
    ad = np.abs(d)
    max_exact = nb // 2  # 8
    is_small = ad < max_exact
    rp = np.maximum(ad, 1).astype(np.float32)
    ril = max_exact + (
        np.log(rp / np.float32(max_exact))
        / np.float32(math.log(MAX_DISTANCE / max_exact))
        * np.float32(nb - max_exact)
    ).astype(np.int32)
    ril = np.minimum(ril, nb - 1)
    return rb + np.where(is_small, ad, ril)


def _near_bucket_tables():
    """Bucket index tile [128, 512] for each near offset m (head-independent)."""
    tables = {}
    p = np.arange(KT)[:, None]
    j = np.arange(QB)[None, :]
    for m in NEAR_MS:
        d = KT * m + p - j  # d = k - q
        tables[m] = _bucket_np(d)
    return tables


_NEAR_BUCKETS = _near_bucket_tables()


def _prep_core_inputs(c, hidden_states, Wq, Wk, Wv, Wo, rel_emb, xt_by_batch):
    """Batch-split sharding: core c handles batch c//4 and head group c%4
    (4 heads, as two head-pairs hp=0,1). Halves per-core xt traffic vs
    replicating both batches."""
    g = c % 4
    rows = slice(256 * g, 256 * (g + 1))

    def wsl(W):  # [(hp*8+dt), 128, 128] chunks of W[rows].T
        wt = np.ascontiguousarray(W[rows].T)  # [1024, 256]
        return np.ascontiguousarray(
            wt.reshape(8, 128, 2, 128).transpose(2, 0, 1, 3)
            .reshape(16, 128, 128)
        ).astype(BF16)

    wqt, wkt, wvt = wsl(Wq), wsl(Wk), wsl(Wv)
    # [2, 128, 1024]: head-pair hp's slice of Wo
    wot = np.ascontiguousarray(
        Wo[:, rows].T.reshape(2, 128, 1024)
    ).astype(BF16)

    # exp(bias) tiles for 4 heads: [(hp*2+hl)*6 + mi, 128, 512]
    etab = np.empty((4 * len(NEAR_MS), KT, QB), dtype=BF16)
    bfar = np.empty((8,), dtype=np.float32)
    for hp in range(2):
        for hl in range(2):
            h = 4 * g + 2 * hp + hl
            for mi, m in enumerate(NEAR_MS):
                etab[(hp * 2 + hl) * len(NEAR_MS) + mi] = np.exp(
                    rel_emb[_NEAR_BUCKETS[m], h].astype(np.float32)
                ).astype(BF16)
            bfar[(hp * 2 + hl) * 2 + 0] = rel_emb[15, h]  # far negative
            bfar[(hp * 2 + hl) * 2 + 1] = rel_emb[31, h]  # far positive
    bfar_t = np.tile(bfar[None, :], (128, 1)).astype(np.float32)
    bfarexp_t = np.exp(bfar_t).astype(np.float32)

    return {
        "xt": xt_by_batch[c // 4],
        "wqt": wqt,
        "wkt": wkt,
        "wvt": wvt,
        "wot": wot,
        "etab": etab,
        "bfar": bfar_t,
        "bfarexp": bfarexp_t,
    }


_PROGRAM_CACHE = {}

# etab DMA order: tiles needed by stream (0,0)'s first kps go first
_ETAB_ORDER = ([1, 2, 7, 8, 3, 4, 9, 10, 5, 11, 0, 6]
               + [13, 14, 19, 20, 15, 16, 21, 22, 17, 23, 12, 18])


def _build_program():
    if "nc" in _PROGRAM_CACHE:
        return _PROGRAM_CACHE["nc"]

    from contextlib import ExitStack

    import concourse.bass as bass
    import concourse.tile as tile
    from concourse import bacc, mybir

    f32 = mybir.dt.float32
    bf16 = mybir.dt.bfloat16
    Exp = mybir.ActivationFunctionType.Exp

    nc = bacc.Bacc("TRN2", target_bir_lowering=False, debug=False,
                   num_devices=N_CORES)

    xt_d = nc.dram_tensor("xt", [N_QB * 8, 128, QB], bf16,
                          kind="ExternalInput").ap()
    wqt_d = nc.dram_tensor("wqt", [16, 128, 128], bf16, kind="ExternalInput").ap()
    wkt_d = nc.dram_tensor("wkt", [16, 128, 128], bf16, kind="ExternalInput").ap()
    wvt_d = nc.dram_tensor("wvt", [16, 128, 128], bf16, kind="ExternalInput").ap()
    wot_d = nc.dram_tensor("wot", [2, 128, 1024], bf16, kind="ExternalInput").ap()
    etab_d = nc.dram_tensor("etab", [24, 128, 512], bf16, kind="ExternalInput").ap()
    bfar_d = nc.dram_tensor("bfar", [128, 8], f32, kind="ExternalInput").ap()
    bfarexp_d = nc.dram_tensor("bfarexp", [128, 8], f32,
                               kind="ExternalInput").ap()
    # per-head-pair partials of this core's batch; host sums them
    out_d = nc.dram_tensor("out", [2, S, D], bf16, kind="ExternalOutput").ap()

    VSLOT = 2 * 65  # [VA | 1 | VB | 1] per (b, kt)

    with tile.TileContext(nc) as tc, ExitStack() as ctx:
        const = ctx.enter_context(tc.tile_pool(name="const", bufs=1))

        xt_sb = const.tile([128, 8 * S], bf16, tag="xt")
        wq_sb = const.tile([128, 16 * 128], bf16, tag="wq")
        wk_sb = const.tile([128, 16 * 128], bf16, tag="wk")
        wv_sb = const.tile([128, 16 * 128], bf16, tag="wv")
        etab_sb = const.tile([128, 24 * 512], bf16, tag="etab")
        bfar_sb = const.tile([128, 8], f32, tag="bfar")
        bfarexp_sb = const.tile([128, 8], f32, tag="bfarexp")
        wot_sb = const.tile([128, 2 * 1024], bf16, tag="wot")

        # DMA priority order: weights, then b0 xt qb0-1, the small bias
        # tables, b0 xt qb2-3, etab, wot, then all of b1's xt.
        def emit_w_dma(hps):
            for hp in hps:
                for w_sb, w_d in ((wk_sb, wkt_d), (wq_sb, wqt_d),
                                  (wv_sb, wvt_d)):
                    for i in range(8):
                        ci = hp * 8 + i
                        eng = (nc.sync, nc.gpsimd, nc.scalar)[i % 3]
                        eng.dma_start(
                            w_sb[:, 128 * ci : 128 * (ci + 1)], w_d[ci])

        def emit_xt_dma(qbs, n_engs=2):
            # scalar may only carry early transfers: later issues on its
            # queue would block the exp stream behind DMA-ring backpressure
            engs = (nc.sync, nc.gpsimd, nc.scalar)[:n_engs]
            for qb in qbs:
                for dt in range(8):
                    eng = engs[dt % n_engs]
                    ci = qb * 8 + dt
                    col = dt * S + qb * QB
                    eng.dma_start(xt_sb[:, col : col + QB], xt_d[ci])

        emit_w_dma((0,))
        emit_xt_dma((0, 1), n_engs=3)
        nc.scalar.dma_start(bfar_sb[:], bfar_d[:])
        nc.scalar.dma_start(bfarexp_sb[:], bfarexp_d[:])
        for j, i in enumerate(_ETAB_ORDER[:8]):
            eng = (nc.sync, nc.gpsimd, nc.scalar)[j % 3]
            eng.dma_start(etab_sb[:, 512 * i : 512 * (i + 1)], etab_d[i])
        emit_xt_dma((2, 3))
        for j, i in enumerate(_ETAB_ORDER[8:12]):
            eng = (nc.sync, nc.gpsimd)[j % 2]
            eng.dma_start(etab_sb[:, 512 * i : 512 * (i + 1)], etab_d[i])
        emit_w_dma((1,))
        for j, i in enumerate(_ETAB_ORDER[12:]):
            eng = (nc.sync, nc.gpsimd)[j % 2]
            eng.dma_start(etab_sb[:, 512 * i : 512 * (i + 1)], etab_d[i])
        for hp in range(2):
            nc.sync.dma_start(wot_sb[:, hp * 1024 : (hp + 1) * 1024],
                              wot_d[hp])

        qt_sb = const.tile([128, B * S], bf16, tag="qt")
        kt_sb = const.tile([128, B * S], bf16, tag="kt")
        v_sb = const.tile([128, B * N_KT * VSLOT], bf16, tag="v")
        # ones columns at offsets 64/129 of each VSLOT: set the whole tile
        # to 1.0 once; the V projection copies overwrite the data columns.
        nc.vector.memset(v_sb[:], 1.0)

        def cls(m):
            if m <= -2:
                return 0  # far negative
            if m >= 5:
                return 1  # far positive
            return 2  # near

        psum = ctx.enter_context(
            tc.tile_pool(name="psum", bufs=2, space="PSUM"))
        utp = ctx.enter_context(tc.tile_pool(name="utp", bufs=8))
        ostg = ctx.enter_context(tc.tile_pool(name="ostg", bufs=4))
        nrm = ctx.enter_context(tc.tile_pool(name="nrm", bufs=2))

        def emit_qk_chain(b, qb, wi):
            w_sb, dst = ((wq_sb, qt_sb), (wk_sb, kt_sb))[wi]
            ps = psum.tile([128, 512], f32, tag="proj",
                           name=f"pj_{b}_{qb}_{wi}")
            for dt in range(8):
                ci = b * 8 + dt
                nc.tensor.matmul(
                    ps[:],
                    lhsT=w_sb[:, 128 * ci : 128 * (ci + 1)],
                    rhs=xt_sb[:, dt * S + qb * QB :
                              dt * S + qb * QB + QB],
                    start=(dt == 0), stop=(dt == 7),
                )
            nc.vector.tensor_copy(
                dst[:, b * S + qb * QB : b * S + qb * QB + QB], ps[:]
            )

        def emit_v_chain(b, st):
            ps = psum.tile([128, 128], f32, tag="proj",
                           name=f"vp_{b}_{st}")
            for dt in range(8):
                ci = b * 8 + dt
                nc.tensor.matmul(
                    ps[:],
                    lhsT=xt_sb[:, dt * S + st * KT :
                               dt * S + st * KT + KT],
                    rhs=wv_sb[:, 128 * ci : 128 * (ci + 1)],
                    start=(dt == 0), stop=(dt == 7),
                )
            base = (b * N_KT + st) * VSLOT
            nc.vector.tensor_copy(v_sb[:, base : base + 64], ps[:, 0:64])
            nc.vector.tensor_copy(v_sb[:, base + 65 : base + 129],
                                  ps[:, 64:128])

        def emit_scores(b, qb, kp):
            """Score matmuls + exp (+ bias mul) for one kp of one stream.
            Returns per-hl source list for the AV matmuls."""
            m0 = 2 * kp - 4 * qb
            m1 = m0 + 1
            c0, c1 = cls(m0), cls(m1)
            out = []
            for hl in range(2):
                sct = psum.tile([128, 1024], f32, tag="sc",
                                name=f"sc_{b}_{qb}_{kp}_{hl}")
                for half in range(2):
                    kt = 2 * kp + half
                    nc.tensor.matmul(
                        sct[:, 512 * half : 512 * (half + 1)],
                        lhsT=kt_sb[64 * hl : 64 * (hl + 1),
                                   b * S + kt * KT : b * S + kt * KT + KT],
                        rhs=qt_sb[64 * hl : 64 * (hl + 1),
                                  b * S + qb * QB : b * S + qb * QB + QB],
                        start=True, stop=True,
                    )
                ut = utp.tile([128, 1024], bf16, tag="ut",
                              name=f"ut_{b}_{qb}_{kp}_{hl}")
                srcs = [(ut, 0), (ut, 512)]
                hg = b * 2 + hl  # head index within this core's 4 heads
                if c0 == c1 and c0 != 2:
                    nc.scalar.activation(
                        ut[:], sct[:], Exp,
                        bias=bfar_sb[:, 2 * hg + c0 : 2 * hg + c0 + 1],
                        scale=SCALING,
                    )
                else:
                    nc.scalar.activation(
                        ut[:], sct[:], Exp, bias=0.0, scale=SCALING
                    )
                    ut2 = utp.tile([128, 1024], bf16, tag="ut2",
                                   name=f"ut2_{b}_{qb}_{kp}_{hl}")
                    if c0 == c1 == 2:
                        ei = (hg * 6 + (m0 + 1)) * 512
                        nc.vector.tensor_mul(
                            ut2[:], ut[:], etab_sb[:, ei : ei + 1024]
                        )
                        srcs = [(ut2, 0), (ut2, 512)]
                    else:
                        for half, (m, cc) in enumerate(((m0, c0), (m1, c1))):
                            usl = ut[:, 512 * half : 512 * (half + 1)]
                            osl = ut2[:, 512 * half : 512 * (half + 1)]
                            if cc == 2:
                                ei = (hg * 6 + (m + 1)) * 512
                                nc.vector.tensor_mul(
                                    osl, usl, etab_sb[:, ei : ei + 512]
                                )
                            else:
                                col = 2 * hg + cc
                                nc.vector.tensor_scalar_mul(
                                    osl, usl, bfarexp_sb[:, col : col + 1]
                                )
                            srcs[half] = (ut2, 512 * half)
                out.append(srcs)
            return out

        def emit_av(b, qb, kp, srcs2, ctxs, hl):
            for half in range(2):
                kt = 2 * kp + half
                base = (b * N_KT + kt) * VSLOT + 65 * hl
                stile, soff = srcs2[hl][half]
                nc.tensor.matmul(
                    ctxs[hl][:],
                    lhsT=v_sb[:, base : base + 65],
                    rhs=stile[:, soff : soff + 512],
                    start=(kt == 0), stop=(kt == N_KT - 1),
                )

        lct_of = {}

        def emit_norm(si):
            b, qb = STREAMS[si]
            ctxs = ctx_of[si]
            lct = nrm.tile([128, 512], bf16, tag="lct", bufs=2,
                           name=f"lct_{b}_{qb}")
            rzfs, rzbs = [], []
            for hl in range(2):
                rz = nrm.tile([1, 512], f32, tag=f"rz{hl}", bufs=2,
                              name=f"rz_{b}_{qb}_{hl}")
                nc.vector.tensor_copy(rz[0:1, :], ctxs[hl][64:65, :])
                rzf = nrm.tile([1, 512], f32, tag=f"rzf{hl}", bufs=2,
                               name=f"rzf_{b}_{qb}_{hl}")
                nc.vector.reciprocal_approx_fast(
                    out=rzf[0:1, :], in_=rz[0:1, :]
                )
                rzfs.append(rzf)
            for hl in range(2):
                rzb = nrm.tile([64, 512], f32, tag=f"rzb{hl}", bufs=2,
                               name=f"rzb_{b}_{qb}_{hl}")
                nc.gpsimd.partition_broadcast(
                    rzb[:], rzfs[hl][0:1, :], channels=64
                )
                rzbs.append(rzb)
            for hl in range(2):
                nc.vector.tensor_mul(
                    lct[64 * hl : 64 * (hl + 1), :],
                    ctxs[hl][0:64, :], rzbs[hl][:])
            lct_of[si] = lct

        def emit_wo(si, st, tail=False):
            b, qb = STREAMS[si]
            lct = lct_of[si]
            ot = ostg.tile([128, 1024], bf16, tag="ot",
                           name=f"ot_{b}_{qb}_{st}")
            for nh in range(2):
                wo_ps = psum.tile([128, 512], f32, tag="proj",
                                  name=f"wo_{b}_{qb}_{st}_{nh}")
                nc.tensor.matmul(
                    wo_ps[:],
                    lhsT=lct[:, st * 128 : (st + 1) * 128],
                    rhs=wot_sb[:, b * 1024 + nh * 512 :
                               b * 1024 + (nh + 1) * 512],
                    start=True, stop=True,
                )
                # in the tail the exps are done, so the scalar engine is
                # free to halve the PSUM-evacuation latency
                if tail and nh == 0:
                    nc.scalar.copy(ot[:, 0:512], wo_ps[:])
                else:
                    nc.vector.tensor_copy(ot[:, nh * 512 : (nh + 1) * 512],
                                          wo_ps[:])
            srow = qb * QB + st * 128
            eng = (nc.gpsimd, nc.sync, nc.scalar)[st % 3] if tail else \
                (nc.gpsimd, nc.sync)[st % 2]
            eng.dma_start(out_d[b, srow : srow + 128, :], ot[:])

        # ---- schedule ----
        STREAMS = [(0, 0), (0, 1), (0, 2), (0, 3),
                   (1, 0), (1, 1), (1, 2), (1, 3)]
        ctx_of = {}

        def qk(b, qb, wi):
            return lambda: emit_qk_chain(b, qb, wi)

        def vch(b, st):
            return lambda: emit_v_chain(b, st)

        def wo(si, st):
            return lambda: emit_wo(si, st)

        def dum(n):
            # p-state filler: keeps the PE streaming through DMA-paced
            # stretches so the clock stays ramped; results are discarded
            def f():
                d = psum.tile([128, 512], f32, tag="proj", name="dum")
                for _ in range(n):
                    nc.tensor.matmul(
                        d[:], lhsT=qt_sb[:, 0:128], rhs=qt_sb[:, 0:512],
                        start=True, stop=True,
                    )
            return f

        fill = {s: {kp: [] for kp in range(8)} for s in range(8)}
        # stream 0: rest of batch-0 projections, need-ordered
        fill[0][0] = [qk(0, 1, 1), vch(0, 2), vch(0, 3)]
        fill[0][1] = [vch(0, 4), vch(0, 5)]
        fill[0][2] = [qk(0, 2, 1), vch(0, 6), vch(0, 7)]
        fill[0][3] = [vch(0, 8), vch(0, 9)]
        fill[0][4] = [qk(0, 3, 1), vch(0, 10), vch(0, 11)]
        fill[0][5] = [vch(0, 12), vch(0, 13), qk(0, 1, 0)]
        fill[0][6] = [vch(0, 14), vch(0, 15)]
        fill[0][7] = [qk(0, 2, 0)]
        # streams 1-3: batch-1 projections + prev stream's norm/wo
        # Wo filler split: st0/st1 land mid-next-stream; st2/st3 land at the
        # FOLLOWING stream's transition slots (lct long since ready there,
        # so they give the PE real work while the fresh norm chain runs).
        fill[1][0] = [qk(0, 3, 0)]
        fill[1][1] = [qk(1, 0, 1)]
        fill[1][2] = [wo(0, 0), vch(1, 0)]
        fill[1][3] = [wo(0, 1), vch(1, 1)]
        fill[1][4] = [wo(0, 2), qk(1, 0, 0)]
        fill[1][5] = [wo(0, 3), vch(1, 2)]
        fill[1][6] = [qk(1, 1, 1)]
        fill[1][7] = [vch(1, 3)]
        fill[2][0] = [vch(1, 4)]
        fill[2][1] = [qk(1, 2, 1), vch(1, 5)]
        fill[2][2] = [wo(1, 0), vch(1, 6)]
        fill[2][3] = [wo(1, 1), vch(1, 7)]
        fill[2][4] = [wo(1, 2), vch(1, 8)]
        fill[2][5] = [wo(1, 3), vch(1, 9)]
        fill[2][6] = [qk(1, 3, 1), vch(1, 10)]
        fill[2][7] = [vch(1, 11)]
        fill[3][0] = [vch(1, 12)]
        fill[3][1] = [vch(1, 13)]
        fill[3][2] = [wo(2, 0), vch(1, 14)]
        fill[3][3] = [wo(2, 1), vch(1, 15)]
        fill[3][4] = [wo(2, 2)]
        fill[3][5] = [wo(2, 3)]
        for s in range(4, 8):
            if s < 7:
                fill[s][0] = [qk(1, s - 3, 0)]
            for st in range(4):
                fill[s][2 + st].append(wo(s - 1, st))
        # streams 4-7 run out of real projection filler: their PE load/slot
        # (~1.9us) sits just under the scalar exp pace (~2.15us), so every
        # slot ends in a micro-gap that resets the PE clock ramp and the
        # attention matmuls drop to mid p-state. Two dummy matmuls per
        # empty slot (~0.43us) pack the slots to the scalar pace and keep
        # the clock pinned at max.
        fill[3][6].append(dum(2))
        fill[3][7].append(dum(2))
        for s in range(4, 8):
            fill[s][1].append(dum(2))
            fill[s][6].append(dum(2))
            fill[s][7].append(dum(2))
        fill[7][0].append(dum(3))

        # prologue: minimum chains for stream (0,0) kp0
        emit_qk_chain(0, 0, 1)  # k
        emit_qk_chain(0, 0, 0)  # q
        emit_v_chain(0, 0)
        emit_v_chain(0, 1)

        all_slots = [(si, b, qb, kp)
                     for si, (b, qb) in enumerate(STREAMS)
                     for kp in range(8)]
        pend = emit_scores(0, 0, 0)
        for i, (si, b, qb, kp) in enumerate(all_slots):
            cur = pend
            if i + 1 < len(all_slots):
                si2, b2, qb2, kp2 = all_slots[i + 1]
                pend = emit_scores(b2, qb2, kp2)
            if kp == 0:
                ctx_of[si] = {
                    hl: psum.tile([65, 512], f32, tag="ctx",
                                  name=f"ctx_{b}_{qb}_{hl}")
                    for hl in range(2)
                }
            for f in fill[si][kp]:
                f()
            emit_av(b, qb, kp, cur, ctx_of[si], 0)
            emit_av(b, qb, kp, cur, ctx_of[si], 1)
            # start the softmax-normalization chain the moment the
            # stream's ctx accumulation completes
            if kp == 7:
                emit_norm(si)

        # tail: final output projections; a few dummy matmuls bridge the
        # norm(7) latency so the final Wo matmuls run at full clock
        dum(6)()
        for st in range(4):
            emit_wo(7, st, tail=True)

    nc.compile()
    _PROGRAM_CACHE["nc"] = nc
    return nc


def run(inputs, trace=False, trace_kwargs=None):
    """Returns (full_output, BassKernelResults)."""
    from concourse.bass_utils import run_bass_kernel_spmd

    hidden_states = np.asarray(inputs["hidden_states"], dtype=np.float32)
    Wq = np.asarray(inputs["Wq"], dtype=np.float32)
    Wk = np.asarray(inputs["Wk"], dtype=np.float32)
    Wv = np.asarray(inputs["Wv"], dtype=np.float32)
    Wo = np.asarray(inputs["Wo"], dtype=np.float32)
    rel_emb = np.asarray(inputs["rel_emb"], dtype=np.float32)

    xt = np.ascontiguousarray(hidden_states.transpose(0, 2, 1))  # [B, D, S]
    # per-batch [(qb, dt), 128, QB] chunk layout matching the kernel's DMA
    xt_by_batch = [
        np.ascontiguousarray(
            xt[b].reshape(8, 128, S // QB, QB).transpose(2, 0, 1, 3)
            .reshape(N_QB * 8, 128, QB)
        ).astype(BF16)
        for b in range(B)
    ]

    nc = _build_program()
    in_maps = [
        _prep_core_inputs(c, hidden_states, Wq, Wk, Wv, Wo, rel_emb,
                          xt_by_batch)
        for c in range(N_CORES)
    ]
    res = run_bass_kernel_spmd(
        nc, in_maps, list(range(N_CORES)), trace=trace,
        **(trace_kwargs or {}),
    )
    out = np.zeros((B, S, D), dtype=np.float32)
    for c in range(N_CORES):
        p = res.results[c]["out"].astype(np.float32)
        out[c // 4] += p[0] + p[1]
    return out, res


def kernel(**inputs):
    out, _ = run(inputs)
    return out
